# revision 64
# baseline (speedup 1.0000x reference)
"""Trainium2 Bass kernel for the LocalGNOBlock (windowed GNN message passing).

Math restructuring (vs the naive 12x full MLP evaluations):
  msg first layer is linear over concat([h_i, h_j, dc]):
      z_d[i] = (A - C)[i] + (B + C)[i+d] + b1,  d in {+-1..+-6}
  where A = h @ W1a, B = h @ W1b, C = coord x w1c (rank-1).
  The msg second layer AND the update first layer's agg branch are fused:
  agg is only consumed by agg @ U1b, so for interior tokens (count == 12)
      u_pre = h @ U1a + sum_d silu(z_d) @ (W2/12 @ U1b) + bias_u
  accumulates as one 13-matmul PSUM group (no agg materialization at all).
  Boundary chunks (first/last 6 tokens need 12/count fixup) keep the
  two-step path.  LayerNorm stats are computed with band-select matmuls
  (channel dim lives on partitions); rstd = exp(-0.5*ln(var+eps)) on ACT;
  the normalize uses rank-1 grids P1 = g x r, P2 = g x (mu*r) - b x 1.

Pipeline: iteration i emits [load(i+4), phase_a(i+3), phase_e(i+1),
phase_x(i-1), phase_m(i)] so silu(c) (5.4us on ACT, the pacing engine)
completes a full iteration before the matmuls that consume it, and the
s2-dependent x/stats matmuls never block the next chunk's d/e matmuls in
the PE's in-order stream.  Steady-state period ~5.9-6.1us/chunk = the ACT
floor.  Engine balance per chunk: ACT = silu 5.4 + s2 0.7; DVE = z-adds
3.8 + D_A/e casts + x-stt + x2; PE = 20 matmuls; GPSIMD compute idle (it
shares the SBUF port with the DVE - anything on it slows the z-adds) but
its SWDGE queue carries the D_B shift DMAs (AXI port, no engine
contention).  Startup DMAs are spread across the sync/scalar/gpsimd
trigger queues, and a scratch-matmul burst warms the HAM clock gate while
the first h chunks stream in.  The pass-2 tail is a 4-engine chain
(row-DMA -> rank-1 grids on PE -> ScalarE PSUM->SBUF copy -> two DVE ops
-> store) pipelined 4 deep by rotating grids through the pass-1 PSUM
banks that are dead in the tail.

Sharding: batch dim B=8 -> one batch element per NeuronCore.
"""

import numpy as np

K = 6
HID = 128
N = 16384
B = 8
EPS = 1e-5
T = 512                 # token chunk (matmul + elementwise granularity)
NCH = N // T            # 32 chunks
OFF0 = 8                # D_full column of token 0 (even, for alignment)
NCOL = N + 2 * OFF0     # D_full width

# offsets ordered in 4 stride-2 groups: (even uses D_A, odd uses D_B)
NEG_EVEN = [-6, -4, -2]
NEG_ODD = [-5, -3, -1]
POS_ODD = [1, 3, 5]
POS_EVEN = [2, 4, 6]
SEG_ORDER = NEG_EVEN + NEG_ODD + POS_ODD + POS_EVEN  # 12 segments in Z

_compiled = None


def _build_bass(dt_act):
    import concourse.bacc as bacc
    import concourse.bass as bass
    import concourse.tile as tile
    from concourse import mybir

    f32 = mybir.dt.float32
    DT = dt_act

    nc = bacc.Bacc("TRN2", target_bir_lowering=False, debug=False)

    # ---- DRAM I/O ----
    hT = nc.dram_tensor("hT", [HID, N], DT, kind="ExternalInput")
    coordR = nc.dram_tensor("coordR", [1, N], DT, kind="ExternalInput")
    W1a = nc.dram_tensor("W1a", [HID, HID], DT, kind="ExternalInput")
    W1b = nc.dram_tensor("W1b", [HID, HID], DT, kind="ExternalInput")
    w1c = nc.dram_tensor("w1c", [1, HID], DT, kind="ExternalInput")      # +w1c
    w1cn = nc.dram_tensor("w1cn", [1, HID], DT, kind="ExternalInput")    # -w1c
    W2s = nc.dram_tensor("W2s", [HID, HID], DT, kind="ExternalInput")     # W2/12
    W2U = nc.dram_tensor("W2U", [HID, HID], DT, kind="ExternalInput")     # W2/12 @ U1b
    U1a = nc.dram_tensor("U1a", [HID, HID], DT, kind="ExternalInput")
    U1b = nc.dram_tensor("U1b", [HID, HID], DT, kind="ExternalInput")
    U2 = nc.dram_tensor("U2", [HID, HID], DT, kind="ExternalInput")
    b1c = nc.dram_tensor("b1c", [HID, 1], f32, kind="ExternalInput")      # msg_b1
    buc = nc.dram_tensor("buc", [HID, 1], f32, kind="ExternalInput")      # upd_b1 + b2@U1b
    b2c = nc.dram_tensor("b2c", [HID, 1], f32, kind="ExternalInput")      # upd_b2 col
    g_row = nc.dram_tensor("g_row", [1, HID], DT, kind="ExternalInput")  # ln_g
    nb_row = nc.dram_tensor("nb_row", [1, HID], DT, kind="ExternalInput")  # -ln_b
    fixf = nc.dram_tensor("fixf", [1, K], f32, kind="ExternalInput")      # 12/count head
    fixl = nc.dram_tensor("fixl", [1, K], f32, kind="ExternalInput")      # 12/count tail
    # band-select matrix: column 63 = 1/128, else 0 (stats row packing)
    selb = nc.dram_tensor("selb", [HID, 2 * 2 * NCH - 1], DT, kind="ExternalInput")
    outT = nc.dram_tensor("outT", [HID, N], DT, kind="ExternalOutput")
    # DRAM bounce rows for the pass-2 broadcast loads (SBUF sources cannot
    # have a stride-0 partition AP, DRAM sources can)
    rN = nc.dram_tensor("rN", [1, N], DT, kind="Internal")
    uN = nc.dram_tensor("uN", [1, N], DT, kind="Internal")

    Silu = mybir.ActivationFunctionType.Silu
    Log = mybir.ActivationFunctionType.Ln
    Exp = mybir.ActivationFunctionType.Exp

    with tile.TileContext(nc) as tc:
        with (
            tc.tile_pool(name="singles", bufs=1) as singles,
            tc.tile_pool(name="big", bufs=1) as big,
            tc.tile_pool(name="work", bufs=2) as work,
            tc.tile_pool(name="zpool", bufs=3) as zpool,
            tc.tile_pool(name="opool", bufs=2) as opool,
            tc.tile_pool(name="psA", bufs=1, space="PSUM") as psA,
            tc.tile_pool(name="psB", bufs=1, space="PSUM") as psB,
            tc.tile_pool(name="psS", bufs=1, space="PSUM") as psS,
        ):
            # ---- constants into SBUF ----
            # the tensors phase_a(0)/phase_e(0) need go on the queue FIRST so
            # the pipeline starts as soon as chunk 0 arrives
            sW1a = singles.tile([HID, HID], DT)
            sW1b = singles.tile([HID, HID], DT)
            sW2s = singles.tile([HID, HID], DT)
            sW2U = singles.tile([HID, HID], DT)
            sU1a = singles.tile([HID, HID], DT)
            sU1b = singles.tile([HID, HID], DT)
            sU2 = singles.tile([HID, HID], DT)
            sw1c = singles.tile([1, HID], DT)
            sw1cn = singles.tile([1, HID], DT)
            sb1 = singles.tile([HID, 1], f32)
            sbu = singles.tile([HID, 1], f32)
            sb2 = singles.tile([HID, 1], f32)
            # phase_a needs: W1b, w1c (sync queue); phase_e needs: W1a, w1cn,
            # b1c (scalar queue - ScalarE is a HWDGE engine too and is idle
            # at startup); this leaves the sync queue free for the h loads
            nc.sync.dma_start(out=sW1b, in_=W1b[:, :])
            nc.sync.dma_start(out=sw1c, in_=w1c[:, :])
            nc.scalar.dma_start(out=sW1a, in_=W1a[:, :])
            nc.scalar.dma_start(out=sw1cn, in_=w1cn[:, :])
            nc.scalar.dma_start(out=sb1, in_=b1c[:, :])

            def load_late_consts():
                # everything first needed from phase_m(0) onwards, on the
                # scalar queue which idles until the first silu
                for sb, dr in [(sW2s, W2s), (sW2U, W2U),
                               (sU1a, U1a), (sU1b, U1b), (sU2, U2)]:
                    nc.scalar.dma_start(out=sb, in_=dr[:, :])
                nc.scalar.dma_start(out=sbu, in_=buc[:, :])
                nc.scalar.dma_start(out=sb2, in_=b2c[:, :])
            # broadcast [1,6] -> [128,6] fix tiles
            sfixf = singles.tile([HID, K], f32)
            sfixl = singles.tile([HID, K], f32)
            def bcast_rows(dr):
                a = dr[0:1, :]
                return bass.AP(tensor=a.tensor, offset=a.offset,
                               ap=[[0, HID]] + list(a.ap[1:]))

            def load_fix_consts():
                # broadcast loads must use the gpsimd SWDGE queue (HWDGE
                # rejects stride-0 partition APs); emitted after the D_B
                # copies for chunks 0-2 so those aren't queued behind them
                nc.gpsimd.dma_start(out=sfixf, in_=bcast_rows(fixf))
                nc.gpsimd.dma_start(out=sfixl, in_=bcast_rows(fixl))
            ssel = singles.tile([HID, 2 * 2 * NCH - 1], DT)
            # [-b ; g] stacked lhsT and [ones ; uu] stacked rhs let p2 be a
            # single K=2 matmul in the tail
            sgnb = singles.tile([2, HID], DT)
            sg = singles.tile([1, HID], DT)

            def load_tail_consts():
                nc.scalar.dma_start(out=ssel, in_=selb[:, :])
                nc.scalar.dma_start(out=sgnb[0:1, :], in_=nb_row[:, :])
                nc.scalar.dma_start(out=sgnb[1:2, :], in_=g_row[:, :])
                nc.scalar.dma_start(out=sg, in_=g_row[:, :])

            # ---- big persistent buffers ----
            h_full = big.tile([HID, N], DT)
            D_A = big.tile([HID, NCOL], DT)      # token j at col OFF0 + j
            D_B = big.tile([HID, NCOL], DT)      # token j at col OFF0 + 1 + j
            x_full = big.tile([HID, N], DT)
            # zero halo columns of D so boundary silu stays finite
            nc.vector.memset(D_A[:, 0:OFF0], 0.0)
            nc.vector.memset(D_A[:, OFF0 + N:NCOL], 0.0)
            nc.vector.memset(D_B[:, 0:OFF0 + 1], 0.0)
            nc.vector.memset(D_B[:, OFF0 + 1 + N:NCOL], 0.0)

            # LN stats: rows [0:32] = E[x]/chunk, [32:64] = E[x^2]/chunk
            st_ps = psS.tile([2 * NCH, T], f32)

            crd = {}
            zs = {}
            s2s = {}

            def ht_of(c):
                return h_full[:, c * T:(c + 1) * T]

            def load_chunk(c, eng=None):
                q = eng if eng is not None else nc.sync
                q.dma_start(out=h_full[:, c * T:(c + 1) * T],
                            in_=hT[:, c * T:(c + 1) * T])
                co = work.tile([1, T], DT, tag="co", bufs=5)
                q.dma_start(out=co, in_=coordR[:, c * T:(c + 1) * T])
                crd[c] = co

            def phase_a(c):
                # D chunk = W1b.T @ h  +  w1c x coord   (PSUM accumulate)
                d_ps = psA.tile([HID, T], f32, tag="d", bufs=1)
                nc.tensor.matmul(d_ps, sW1b, ht_of(c), start=True, stop=False)
                nc.tensor.matmul(d_ps, sw1c, crd[c], start=False, stop=True)
                col = OFF0 + c * T
                nc.vector.tensor_copy(D_A[:, col:col + T], d_ps)
                # shifted copy for odd-offset alignment: DMA uses the AXI
                # port, so it does not contend with DVE/ACT engine ports;
                # the gpsimd queue keeps it off the sync queue's h loads
                nc.gpsimd.dma_start(out=D_B[:, col + 1:col + 1 + T],
                                    in_=D_A[:, col:col + T])

            def seg_in1(tile_ap, col):
                # [128, 3, T] AP over D with outer column-stride 2
                s = tile_ap[:, col:col + T]
                return bass.AP(tensor=s.tensor, offset=s.offset,
                               ap=[s.ap[0], [2, 3], [1, T]])

            def phase_e(t):
                # E chunk = W1a.T @ h - w1c x coord
                e_ps = psA.tile([HID, T], f32, tag="e", bufs=2)
                nc.tensor.matmul(e_ps, sW1a, ht_of(t), start=True, stop=False)
                nc.tensor.matmul(e_ps, sw1cn, crd[t], start=False, stop=True)
                e_sb = work.tile([HID, T], DT, tag="esb", bufs=2)
                nc.vector.tensor_copy(e_sb, e_ps)

                # Z: 12 segments of E + shifted D, 4 stride-2 groups
                z = zpool.tile([HID, 12 * T], DT, tag="z", bufs=3)
                zv = z.rearrange("p (s t) -> p s t", t=T)
                e_b = bass.AP(tensor=e_sb.tensor, offset=e_sb.offset,
                              ap=[e_sb.ap[0], [0, 3], [1, T]])
                base = t * T
                groups = [
                    (D_A, OFF0 + base + NEG_EVEN[0]),
                    (D_B, OFF0 + 1 + base + NEG_ODD[0]),
                    (D_B, OFF0 + 1 + base + POS_ODD[0]),
                    (D_A, OFF0 + base + POS_EVEN[0]),
                ]
                for gi, (dbuf, col) in enumerate(groups):
                    nc.vector.tensor_tensor(
                        out=zv[:, 3 * gi:3 * gi + 3, :],
                        in0=e_b, in1=seg_in1(dbuf, col),
                        op=mybir.AluOpType.add)

                # silu over all 12 segments at once (bias = msg_b1)
                nc.scalar.activation(z, z, Silu, bias=sb1, scale=1.0)

                # zero invalid boundary columns (torn edges of the sequence)
                if t == 0:
                    for s, d in enumerate(SEG_ORDER):
                        if d < 0:
                            nc.vector.memset(zv[:, s, 0:-d], 0.0)
                if t == NCH - 1:
                    for s, d in enumerate(SEG_ORDER):
                        if d > 0:
                            nc.vector.memset(zv[:, s, T - d:T], 0.0)
                zs[t] = z

            def phase_m(t):
                ht = ht_of(t)
                zv = zs[t].rearrange("p (s t) -> p s t", t=T)
                boundary = t == 0 or t == NCH - 1
                u_ps = psA.tile([HID, T], f32, tag="u", bufs=2)
                if boundary:
                    # two-step path so the 12/count fixup can apply to agg
                    a_ps = psB.tile([HID, T], f32, tag="agg", bufs=1)
                    for s in range(12):
                        nc.tensor.matmul(a_ps, sW2s, zv[:, s, :],
                                         start=(s == 0), stop=(s == 11))
                    agg = work.tile([HID, T], DT, tag="agg_sb", bufs=1)
                    nc.vector.tensor_copy(agg, a_ps)
                    if t == 0:
                        nc.vector.tensor_tensor(
                            out=agg[:, 0:K], in0=a_ps[:, 0:K],
                            in1=sfixf, op=mybir.AluOpType.mult)
                    else:
                        nc.vector.tensor_tensor(
                            out=agg[:, T - K:T], in0=a_ps[:, T - K:T],
                            in1=sfixl, op=mybir.AluOpType.mult)
                    nc.tensor.matmul(u_ps, sU1a, ht, start=True, stop=False)
                    nc.tensor.matmul(u_ps, sU1b, agg, start=False, stop=True)
                else:
                    # fused: u_pre = U1a.T@h + sum_s W2U.T@silu(z_s)
                    nc.tensor.matmul(u_ps, sU1a, ht, start=True, stop=False)
                    for s in range(12):
                        nc.tensor.matmul(u_ps, sW2U, zv[:, s, :],
                                         start=False, stop=(s == 11))
                s2 = work.tile([HID, T], DT, tag="s2", bufs=2)
                nc.scalar.activation(s2, u_ps, Silu, bias=sbu, scale=1.0)
                s2s[t] = s2
                del crd[t], zs[t]

            def phase_x(t):
                # deferred one iteration behind phase_m so the s2-dependent
                # x matmul never blocks the next chunk's d/e matmuls in the
                # PE's in-order stream
                ht = ht_of(t)
                # x = h + (U2@s2 + b2): PE computes U2@s2, the DVE fused op
                # adds the per-channel bias and the residual in one pass
                x_ps = psA.tile([HID, T], f32, tag="x", bufs=1)
                nc.tensor.matmul(x_ps, sU2, s2s[t], start=True, stop=True)
                base = t * T
                x_sb = x_full[:, base:base + T]
                nc.vector.scalar_tensor_tensor(
                    out=x_sb, in0=x_ps, scalar=sb2, in1=ht,
                    op0=mybir.AluOpType.add, op1=mybir.AluOpType.add)
                x2 = work.tile([HID, T], DT, tag="x2", bufs=2)
                nc.vector.tensor_tensor(out=x2, in0=x_sb, in1=x_sb,
                                        op=mybir.AluOpType.mult)
                # LN stats rows: band-select lhsT packs E[x] into psum row t
                # and E[x^2] into row NCH+t of one accumulating [64,T] bank
                hot = 2 * NCH - 1
                nc.tensor.matmul(st_ps[:, :], ssel[:, hot - t:hot - t + 2 * NCH],
                                 x_sb, start=(t == 0), stop=False)
                nc.tensor.matmul(st_ps[:, :],
                                 ssel[:, hot - NCH - t:hot - t + NCH],
                                 x2, start=False, stop=(t == NCH - 1))
                if 0 < t < NCH - 1:
                    # tiny keep-warm matmuls: the HAM clock gate re-throttles
                    # the PE after idle stretches; these fill the stall tails
                    # so the array stays at full clock (~135ns each)
                    dmy = psB.tile([HID, HID], f32, tag="agg", bufs=1)
                    nc.tensor.matmul(dmy, sW2s, sW2U, start=True, stop=True)
                    dmy2 = psB.tile([HID, HID], f32, tag="agg", bufs=1)
                    nc.tensor.matmul(dmy2, sW2s, sU1a, start=True, stop=True)
                del s2s[t]

            # ---------------- pass 1 (software-pipelined) ----------------
            # PE warm-up: the first ~14us are DMA-bound while h/weights
            # stream in.  A run of back-to-back scratch matmuls (emitted
            # FIRST, so they sit ahead of all real matmuls in the PE's
            # in-order queue) keeps the HAM activity monitor busy so the
            # clock gate is at 8/8 when the real work arrives.
            scr = work.tile([HID, T], DT, tag="scr", bufs=1)
            nc.vector.memset(scr, 0.0)
            for w in range(24):
                if w % 2 == 0:
                    wm = psB.tile([HID, T], f32, tag="agg", bufs=1)
                else:
                    wm = psA.tile([HID, T], f32, tag="u", bufs=2)
                nc.tensor.matmul(wm, scr[:, 0:HID], scr, start=True, stop=True)
            load_chunk(0)
            load_chunk(1)
            load_chunk(2, eng=nc.scalar)
            load_chunk(3, eng=nc.scalar)
            phase_a(0)
            phase_a(1)
            phase_a(2)
            load_fix_consts()
            load_late_consts()
            load_tail_consts()
            phase_e(0)
            # second warm-up burst: during pipeline fill the PE only has the
            # d/e matmuls of chunks 0-2 (~25% duty) and the HAM re-throttles,
            # making iterations 2-7 run at half clock.  These fillers run in
            # the idle window (they only delay matmuls that wait on silu(0)
            # anyway) and keep the activity monitor busy until steady-state
            # density takes over.
            for w in range(16):
                if w % 2 == 0:
                    wm = psB.tile([HID, T], f32, tag="agg", bufs=1)
                else:
                    wm = psA.tile([HID, T], f32, tag="u", bufs=2)
                nc.tensor.matmul(wm, scr[:, 0:HID], scr, start=True, stop=True)
            for i in range(NCH):
                if i + 4 < NCH:
                    load_chunk(i + 4)
                if i + 3 < NCH:
                    phase_a(i + 3)
                # phase_e before phase_x/phase_m: the z-adds reach the DVE
                # queue ahead of the s2-coupled x ops, so silu(i+1) starts
                # as early as possible on ACT
                if i + 1 < NCH:
                    phase_e(i + 1)
                if i >= 1:
                    phase_x(i - 1)
                phase_m(i)
            phase_x(NCH - 1)

            # ---------------- LN stats math ----------------
            # rstd = exp(-0.5 * log(var + eps)) on ACT (ln+exp share a table set)
            r_sb = big.tile([NCH, T], DT)       # rstd per token
            u_sb = big.tile([NCH, T], DT)       # mu * rstd per token
            ex_sb = work.tile([NCH, T], f32, tag="ex")
            nc.vector.tensor_copy(ex_sb, st_ps[0:NCH, :])
            t1 = work.tile([NCH, T], f32, tag="t1")
            nc.vector.tensor_tensor(out=t1, in0=ex_sb, in1=ex_sb,
                                    op=mybir.AluOpType.mult)
            # in-place from here: t1 -> var -> ln(var+eps)
            nc.vector.tensor_tensor(out=t1, in0=st_ps[NCH:2 * NCH, :], in1=t1,
                                    op=mybir.AluOpType.subtract)
            seps = singles.tile([NCH, 1], f32)
            nc.vector.memset(seps, float(EPS))
            szero = singles.tile([NCH, 1], f32)
            nc.vector.memset(szero, 0.0)
            nc.scalar.activation(t1, t1, Log, bias=seps, scale=1.0)
            with nc.allow_low_precision(reason="rstd rows feed fp16 matmuls"):
                nc.scalar.activation(r_sb, t1, Exp, bias=szero, scale=-0.5)
            nc.vector.tensor_tensor(out=u_sb, in0=ex_sb,
                                    in1=r_sb, op=mybir.AluOpType.mult)
            # ---------------- pass 2: normalize ----------------
            # o = x * P1 - P2 with P1 = g x rstd, P2 = g x (mu*rstd) - b x 1
            # built as rank-1 grids on the PE; K=1/K=2 matmul rhs rows are
            # DMA'd down to partition 0 first.
            for t in range(NCH):
                base = t * T
                rr = work.tile([1, T], DT, tag="rr", bufs=4)
                nc.gpsimd.dma_start(out=rr, in_=r_sb[t:t + 1, :])
                uo = work.tile([2, T], DT, tag="uo", bufs=4)
                if t < 4:
                    nc.vector.memset(uo[0:1, :], 1.0)
                nc.sync.dma_start(out=uo[1:2, :], in_=u_sb[t:t + 1, :])
                # rotate through the pass-1 psum banks that are dead in the
                # tail for 4-deep grid pipelining (the chain spans 4 engines,
                # ~4.8us of latency per chunk)
                if t % 4 == 2:
                    p1 = psA.tile([HID, T], f32, tag="d", bufs=1)
                elif t % 4 == 3:
                    p1 = psB.tile([HID, T], f32, tag="agg", bufs=1)
                else:
                    p1 = psA.tile([HID, T], f32, tag="e", bufs=2)
                if t % 2 == 0:
                    p2 = psA.tile([HID, T], f32, tag="u", bufs=2)
                else:
                    p2 = psA.tile([HID, T], f32, tag="x", bufs=1)
                nc.tensor.matmul(p1, sg, rr, start=True, stop=True)
                nc.tensor.matmul(p2, sgnb, uo, start=True, stop=True)
                # p1 -> SBUF via ScalarE so the multiply runs at the fp16 2x
                # DVE rate; the subtract reads p2 from PSUM
                sp1 = work.tile([HID, T], DT, tag="sp1", bufs=4)
                nc.scalar.copy(out=sp1, in_=p1)
                o = opool.tile([HID, T], DT, tag="o", bufs=3)
                nc.vector.tensor_tensor(out=o, in0=x_full[:, base:base + T],
                                        in1=sp1, op=mybir.AluOpType.mult)
                nc.vector.tensor_tensor(out=o, in0=o, in1=p2,
                                        op=mybir.AluOpType.subtract)
                # alternate store queues: gps also carries the rr rows, and
                # the 4MB of output transfers otherwise lag the tail drain
                # (sync only has the uo rows here; scalar would serialize
                # the stores with the sp1 copies)
                if t % 2 == 0:
                    nc.gpsimd.dma_start(out=outT[:, base:base + T], in_=o)
                else:
                    nc.sync.dma_start(out=outT[:, base:base + T], in_=o)

    nc.compile()
    return nc


def _get_compiled(dt_name):
    global _compiled
    if _compiled is None:
        from concourse import mybir
        dt = {"bf16": mybir.dt.bfloat16, "fp16": mybir.dt.float16, "fp32": mybir.dt.float32}[dt_name]
        _compiled = _build_bass(dt)
    return _compiled


DT_NAME = "fp16"


def _sel_band(act_np):
    hot = 2 * NCH - 1
    sel = np.zeros((HID, 2 * 2 * NCH - 1), dtype=np.float32)
    sel[:, hot] = 1.0 / HID
    return sel.astype(act_np)


def kernel(**inputs):
    from concourse.bass_utils import run_bass_kernel_spmd

    h = np.asarray(inputs["h"], dtype=np.float32)
    coord = np.asarray(inputs["coord"], dtype=np.float32)
    msg_w1 = np.asarray(inputs["msg_w1"], dtype=np.float32)
    msg_b1 = np.asarray(inputs["msg_b1"], dtype=np.float32)
    msg_w2 = np.asarray(inputs["msg_w2"], dtype=np.float32)
    msg_b2 = np.asarray(inputs["msg_b2"], dtype=np.float32)
    upd_w1 = np.asarray(inputs["upd_w1"], dtype=np.float32)
    upd_b1 = np.asarray(inputs["upd_b1"], dtype=np.float32)
    upd_w2 = np.asarray(inputs["upd_w2"], dtype=np.float32)
    upd_b2 = np.asarray(inputs["upd_b2"], dtype=np.float32)
    ln_g = np.asarray(inputs["ln_g"], dtype=np.float32)
    ln_b = np.asarray(inputs["ln_b"], dtype=np.float32)

    import ml_dtypes
    act_np = {"bf16": ml_dtypes.bfloat16, "fp16": np.float16, "fp32": np.float32}[DT_NAME]

    W1a = msg_w1[:HID]
    W1b = msg_w1[HID:2 * HID]
    w1c = msg_w1[2 * HID]
    U1b_m = upd_w1[HID:2 * HID]
    bias_u = upd_b1 + msg_b2 @ U1b_m
    W2s = msg_w2 / (2.0 * K)
    W2U = (msg_w2.astype(np.float64) / (2.0 * K) @ U1b_m.astype(np.float64)).astype(np.float32)

    idx = np.arange(N)
    count = (np.minimum(idx, K) + np.minimum(N - 1 - idx, K)).astype(np.float32)
    fix = (2.0 * K) / count
    fixf = fix[:K].reshape(1, K).astype(np.float32)
    fixl = fix[N - K:].reshape(1, K).astype(np.float32)

    const = {
        "W1a": np.ascontiguousarray(W1a, dtype=act_np),
        "W1b": np.ascontiguousarray(W1b, dtype=act_np),
        "w1c": np.ascontiguousarray(w1c.reshape(1, HID), dtype=act_np),
        "w1cn": np.ascontiguousarray(-w1c.reshape(1, HID), dtype=act_np),
        "W2s": np.ascontiguousarray(W2s, dtype=act_np),
        "W2U": np.ascontiguousarray(W2U, dtype=act_np),
        "U1a": np.ascontiguousarray(upd_w1[:HID], dtype=act_np),
        "U1b": np.ascontiguousarray(U1b_m, dtype=act_np),
        "U2": np.ascontiguousarray(upd_w2, dtype=act_np),
        "b1c": np.ascontiguousarray(msg_b1.reshape(HID, 1), dtype=np.float32),
        "buc": np.ascontiguousarray(bias_u.reshape(HID, 1), dtype=np.float32),
        "b2c": np.ascontiguousarray(upd_b2.reshape(HID, 1), dtype=np.float32),
        "g_row": np.ascontiguousarray(ln_g.reshape(1, HID), dtype=act_np),
        "nb_row": np.ascontiguousarray(-ln_b.reshape(1, HID), dtype=act_np),
        "fixf": fixf,
        "fixl": fixl,
        "selb": _sel_band(act_np),
    }

    in_maps = []
    for b in range(B):
        m = dict(const)
        m["hT"] = np.ascontiguousarray(h[b].T, dtype=act_np)
        m["coordR"] = np.ascontiguousarray(coord[b].reshape(1, N), dtype=act_np)
        in_maps.append(m)

    nc = _get_compiled(DT_NAME)
    res = run_bass_kernel_spmd(nc, in_maps, core_ids=list(range(B)))
    global LAST_RESULTS
    LAST_RESULTS = res
    out = np.stack([np.asarray(res.results[b]["outT"], dtype=np.float32).T
                    for b in range(B)])
    return np.ascontiguousarray(out)


# revision 66
# speedup vs baseline: 1.0094x; 1.0094x over previous
"""Trainium2 Bass kernel for the LocalGNOBlock (windowed GNN message passing).

Math restructuring (vs the naive 12x full MLP evaluations):
  msg first layer is linear over concat([h_i, h_j, dc]):
      z_d[i] = (A - C)[i] + (B + C)[i+d] + b1,  d in {+-1..+-6}
  where A = h @ W1a, B = h @ W1b, C = coord x w1c (rank-1).
  The msg second layer AND the update first layer's agg branch are fused:
  agg is only consumed by agg @ U1b, so for interior tokens (count == 12)
      u_pre = h @ U1a + sum_d silu(z_d) @ (W2/12 @ U1b) + bias_u
  accumulates as one 13-matmul PSUM group (no agg materialization at all).
  Boundary chunks (first/last 6 tokens need 12/count fixup) keep the
  two-step path.  LayerNorm stats are computed with band-select matmuls
  (channel dim lives on partitions); rstd = exp(-0.5*ln(var+eps)) on ACT;
  the normalize uses rank-1 grids P1 = g x r, P2 = g x (mu*r) - b x 1.

Pipeline: iteration i emits [load(i+4), phase_a(i+3), phase_e(i+1),
phase_x(i-1), phase_m(i)] so silu(c) (5.4us on ACT, the pacing engine)
completes a full iteration before the matmuls that consume it, and the
s2-dependent x/stats matmuls never block the next chunk's d/e matmuls in
the PE's in-order stream.  Steady-state period ~5.9-6.1us/chunk = the ACT
floor.  Engine balance per chunk: ACT = silu 5.4 + s2 0.7; DVE = z-adds
3.8 + D_A/e casts + x-stt + x2; PE = 20 matmuls; GPSIMD compute idle (it
shares the SBUF port with the DVE - anything on it slows the z-adds) but
its SWDGE queue carries the D_B shift DMAs (AXI port, no engine
contention).  Startup DMAs are spread across the sync/scalar/gpsimd
trigger queues, and a scratch-matmul burst warms the HAM clock gate while
the first h chunks stream in.  The pass-2 tail is a 4-engine chain
(row-DMA -> rank-1 grids on PE -> ScalarE PSUM->SBUF copy -> two DVE ops
-> store) pipelined 4 deep by rotating grids through the pass-1 PSUM
banks that are dead in the tail.

Sharding: batch dim B=8 -> one batch element per NeuronCore.
"""

import numpy as np

K = 6
HID = 128
N = 16384
B = 8
EPS = 1e-5
T = 512                 # token chunk (matmul + elementwise granularity)
NCH = N // T            # 32 chunks
OFF0 = 8                # D_full column of token 0 (even, for alignment)
NCOL = N + 2 * OFF0     # D_full width

# offsets ordered in 4 stride-2 groups: (even uses D_A, odd uses D_B)
NEG_EVEN = [-6, -4, -2]
NEG_ODD = [-5, -3, -1]
POS_ODD = [1, 3, 5]
POS_EVEN = [2, 4, 6]
SEG_ORDER = NEG_EVEN + NEG_ODD + POS_ODD + POS_EVEN  # 12 segments in Z

_compiled = None


def _build_bass(dt_act):
    import concourse.bacc as bacc
    import concourse.bass as bass
    import concourse.tile as tile
    from concourse import mybir

    f32 = mybir.dt.float32
    DT = dt_act

    nc = bacc.Bacc("TRN2", target_bir_lowering=False, debug=False)

    # ---- DRAM I/O ----
    hT = nc.dram_tensor("hT", [HID, N], DT, kind="ExternalInput")
    coordR = nc.dram_tensor("coordR", [1, N], DT, kind="ExternalInput")
    W1a = nc.dram_tensor("W1a", [HID, HID], DT, kind="ExternalInput")
    W1b = nc.dram_tensor("W1b", [HID, HID], DT, kind="ExternalInput")
    w1c = nc.dram_tensor("w1c", [1, HID], DT, kind="ExternalInput")      # +w1c
    w1cn = nc.dram_tensor("w1cn", [1, HID], DT, kind="ExternalInput")    # -w1c
    W2s = nc.dram_tensor("W2s", [HID, HID], DT, kind="ExternalInput")     # W2/12
    W2U = nc.dram_tensor("W2U", [HID, HID], DT, kind="ExternalInput")     # W2/12 @ U1b
    U1a = nc.dram_tensor("U1a", [HID, HID], DT, kind="ExternalInput")
    U1b = nc.dram_tensor("U1b", [HID, HID], DT, kind="ExternalInput")
    U2 = nc.dram_tensor("U2", [HID, HID], DT, kind="ExternalInput")
    b1c = nc.dram_tensor("b1c", [HID, 1], f32, kind="ExternalInput")      # msg_b1
    buc = nc.dram_tensor("buc", [HID, 1], f32, kind="ExternalInput")      # upd_b1 + b2@U1b
    b2c = nc.dram_tensor("b2c", [HID, 1], f32, kind="ExternalInput")      # upd_b2 col
    g_row = nc.dram_tensor("g_row", [1, HID], DT, kind="ExternalInput")  # ln_g
    nb_row = nc.dram_tensor("nb_row", [1, HID], DT, kind="ExternalInput")  # -ln_b
    fixf = nc.dram_tensor("fixf", [1, K], f32, kind="ExternalInput")      # 12/count head
    fixl = nc.dram_tensor("fixl", [1, K], f32, kind="ExternalInput")      # 12/count tail
    # band-select matrix: column 63 = 1/128, else 0 (stats row packing)
    selb = nc.dram_tensor("selb", [HID, 2 * 2 * NCH - 1], DT, kind="ExternalInput")
    outT = nc.dram_tensor("outT", [HID, N], DT, kind="ExternalOutput")
    # DRAM bounce rows for the pass-2 broadcast loads (SBUF sources cannot
    # have a stride-0 partition AP, DRAM sources can)
    rN = nc.dram_tensor("rN", [1, N], DT, kind="Internal")
    uN = nc.dram_tensor("uN", [1, N], DT, kind="Internal")

    Silu = mybir.ActivationFunctionType.Silu
    Log = mybir.ActivationFunctionType.Ln
    Exp = mybir.ActivationFunctionType.Exp

    with tile.TileContext(nc) as tc:
        with (
            tc.tile_pool(name="singles", bufs=1) as singles,
            tc.tile_pool(name="big", bufs=1) as big,
            tc.tile_pool(name="work", bufs=2) as work,
            tc.tile_pool(name="zpool", bufs=3) as zpool,
            tc.tile_pool(name="opool", bufs=2) as opool,
            tc.tile_pool(name="psA", bufs=1, space="PSUM") as psA,
            tc.tile_pool(name="psB", bufs=1, space="PSUM") as psB,
            tc.tile_pool(name="psS", bufs=1, space="PSUM") as psS,
        ):
            # ---- constants into SBUF ----
            # the tensors phase_a(0)/phase_e(0) need go on the queue FIRST so
            # the pipeline starts as soon as chunk 0 arrives
            sW1a = singles.tile([HID, HID], DT)
            sW1b = singles.tile([HID, HID], DT)
            sW2s = singles.tile([HID, HID], DT)
            sW2U = singles.tile([HID, HID], DT)
            sU1a = singles.tile([HID, HID], DT)
            sU1b = singles.tile([HID, HID], DT)
            sU2 = singles.tile([HID, HID], DT)
            sw1c = singles.tile([1, HID], DT)
            sw1cn = singles.tile([1, HID], DT)
            sb1 = singles.tile([HID, 1], f32)
            sbu = singles.tile([HID, 1], f32)
            sb2 = singles.tile([HID, 1], f32)
            # phase_a needs: W1b, w1c (sync queue); phase_e needs: W1a, w1cn,
            # b1c (scalar queue - ScalarE is a HWDGE engine too and is idle
            # at startup); this leaves the sync queue free for the h loads
            nc.sync.dma_start(out=sW1b, in_=W1b[:, :])
            nc.sync.dma_start(out=sw1c, in_=w1c[:, :])
            nc.scalar.dma_start(out=sW1a, in_=W1a[:, :])
            nc.scalar.dma_start(out=sw1cn, in_=w1cn[:, :])
            nc.scalar.dma_start(out=sb1, in_=b1c[:, :])

            def load_late_consts():
                # everything first needed from phase_m(0) onwards, on the
                # scalar queue which idles until the first silu
                for sb, dr in [(sW2s, W2s), (sW2U, W2U),
                               (sU1a, U1a), (sU1b, U1b), (sU2, U2)]:
                    nc.scalar.dma_start(out=sb, in_=dr[:, :])
                nc.scalar.dma_start(out=sbu, in_=buc[:, :])
                nc.scalar.dma_start(out=sb2, in_=b2c[:, :])
            # broadcast [1,6] -> [128,6] fix tiles
            sfixf = singles.tile([HID, K], f32)
            sfixl = singles.tile([HID, K], f32)
            def bcast_rows(dr):
                a = dr[0:1, :]
                return bass.AP(tensor=a.tensor, offset=a.offset,
                               ap=[[0, HID]] + list(a.ap[1:]))

            def load_fix_consts():
                # broadcast loads must use the gpsimd SWDGE queue (HWDGE
                # rejects stride-0 partition APs); emitted after the D_B
                # copies for chunks 0-2 so those aren't queued behind them
                nc.gpsimd.dma_start(out=sfixf, in_=bcast_rows(fixf))
                nc.gpsimd.dma_start(out=sfixl, in_=bcast_rows(fixl))
            ssel = singles.tile([HID, 2 * 2 * NCH - 1], DT)
            # [-b ; g] stacked lhsT and [ones ; uu] stacked rhs let p2 be a
            # single K=2 matmul in the tail
            sgnb = singles.tile([2, HID], DT)
            sg = singles.tile([1, HID], DT)

            def load_tail_consts():
                nc.scalar.dma_start(out=ssel, in_=selb[:, :])
                nc.scalar.dma_start(out=sgnb[0:1, :], in_=nb_row[:, :])
                nc.scalar.dma_start(out=sgnb[1:2, :], in_=g_row[:, :])
                nc.scalar.dma_start(out=sg, in_=g_row[:, :])

            # ---- big persistent buffers ----
            h_full = big.tile([HID, N], DT)
            D_A = big.tile([HID, NCOL], DT)      # token j at col OFF0 + j
            D_B = big.tile([HID, NCOL], DT)      # token j at col OFF0 + 1 + j
            x_full = big.tile([HID, N], DT)
            # zero halo columns of D so boundary silu stays finite
            nc.vector.memset(D_A[:, 0:OFF0], 0.0)
            nc.vector.memset(D_A[:, OFF0 + N:NCOL], 0.0)
            nc.vector.memset(D_B[:, 0:OFF0 + 1], 0.0)
            nc.vector.memset(D_B[:, OFF0 + 1 + N:NCOL], 0.0)

            # LN stats: rows [0:32] = E[x]/chunk, [32:64] = E[x^2]/chunk
            st_ps = psS.tile([2 * NCH, T], f32)

            crd = {}
            zs = {}
            s2s = {}

            def ht_of(c):
                return h_full[:, c * T:(c + 1) * T]

            def load_chunk(c, eng=None):
                q = eng if eng is not None else nc.sync
                q.dma_start(out=h_full[:, c * T:(c + 1) * T],
                            in_=hT[:, c * T:(c + 1) * T])
                co = work.tile([1, T], DT, tag="co", bufs=5)
                q.dma_start(out=co, in_=coordR[:, c * T:(c + 1) * T])
                crd[c] = co

            def phase_a(c):
                # D chunk = W1b.T @ h  +  w1c x coord   (PSUM accumulate)
                d_ps = psA.tile([HID, T], f32, tag="d", bufs=1)
                nc.tensor.matmul(d_ps, sW1b, ht_of(c), start=True, stop=False)
                nc.tensor.matmul(d_ps, sw1c, crd[c], start=False, stop=True)
                col = OFF0 + c * T
                nc.vector.tensor_copy(D_A[:, col:col + T], d_ps)
                # shifted copy for odd-offset alignment: DMA uses the AXI
                # port, so it does not contend with DVE/ACT engine ports;
                # the gpsimd queue keeps it off the sync queue's h loads
                nc.gpsimd.dma_start(out=D_B[:, col + 1:col + 1 + T],
                                    in_=D_A[:, col:col + T])

            def seg_in1(tile_ap, col):
                # [128, 3, T] AP over D with outer column-stride 2
                s = tile_ap[:, col:col + T]
                return bass.AP(tensor=s.tensor, offset=s.offset,
                               ap=[s.ap[0], [2, 3], [1, T]])

            def phase_e(t):
                # E chunk = W1a.T @ h - w1c x coord
                e_ps = psA.tile([HID, T], f32, tag="e", bufs=2)
                nc.tensor.matmul(e_ps, sW1a, ht_of(t), start=True, stop=False)
                nc.tensor.matmul(e_ps, sw1cn, crd[t], start=False, stop=True)
                e_sb = work.tile([HID, T], DT, tag="esb", bufs=2)
                nc.vector.tensor_copy(e_sb, e_ps)

                # Z: 12 segments of E + shifted D, 4 stride-2 groups
                z = zpool.tile([HID, 12 * T], DT, tag="z", bufs=3)
                zv = z.rearrange("p (s t) -> p s t", t=T)
                e_b = bass.AP(tensor=e_sb.tensor, offset=e_sb.offset,
                              ap=[e_sb.ap[0], [0, 3], [1, T]])
                base = t * T
                groups = [
                    (D_A, OFF0 + base + NEG_EVEN[0]),
                    (D_B, OFF0 + 1 + base + NEG_ODD[0]),
                    (D_B, OFF0 + 1 + base + POS_ODD[0]),
                    (D_A, OFF0 + base + POS_EVEN[0]),
                ]
                for gi, (dbuf, col) in enumerate(groups):
                    nc.vector.tensor_tensor(
                        out=zv[:, 3 * gi:3 * gi + 3, :],
                        in0=e_b, in1=seg_in1(dbuf, col),
                        op=mybir.AluOpType.add)

                # silu over all 12 segments at once (bias = msg_b1)
                nc.scalar.activation(z, z, Silu, bias=sb1, scale=1.0)

                # zero invalid boundary columns (torn edges of the sequence)
                if t == 0:
                    for s, d in enumerate(SEG_ORDER):
                        if d < 0:
                            nc.vector.memset(zv[:, s, 0:-d], 0.0)
                if t == NCH - 1:
                    for s, d in enumerate(SEG_ORDER):
                        if d > 0:
                            nc.vector.memset(zv[:, s, T - d:T], 0.0)
                zs[t] = z

            def phase_m(t):
                ht = ht_of(t)
                zv = zs[t].rearrange("p (s t) -> p s t", t=T)
                boundary = t == 0 or t == NCH - 1
                u_ps = psA.tile([HID, T], f32, tag="u", bufs=2)
                if boundary:
                    # two-step path so the 12/count fixup can apply to agg
                    a_ps = psB.tile([HID, T], f32, tag="agg", bufs=1)
                    for s in range(12):
                        nc.tensor.matmul(a_ps, sW2s, zv[:, s, :],
                                         start=(s == 0), stop=(s == 11))
                    agg = work.tile([HID, T], DT, tag="agg_sb", bufs=1)
                    nc.vector.tensor_copy(agg, a_ps)
                    if t == 0:
                        nc.vector.tensor_tensor(
                            out=agg[:, 0:K], in0=a_ps[:, 0:K],
                            in1=sfixf, op=mybir.AluOpType.mult)
                    else:
                        nc.vector.tensor_tensor(
                            out=agg[:, T - K:T], in0=a_ps[:, T - K:T],
                            in1=sfixl, op=mybir.AluOpType.mult)
                    nc.tensor.matmul(u_ps, sU1a, ht, start=True, stop=False)
                    nc.tensor.matmul(u_ps, sU1b, agg, start=False, stop=True)
                else:
                    # fused: u_pre = U1a.T@h + sum_s W2U.T@silu(z_s)
                    nc.tensor.matmul(u_ps, sU1a, ht, start=True, stop=False)
                    for s in range(12):
                        nc.tensor.matmul(u_ps, sW2U, zv[:, s, :],
                                         start=False, stop=(s == 11))
                s2 = work.tile([HID, T], DT, tag="s2", bufs=2)
                nc.scalar.activation(s2, u_ps, Silu, bias=sbu, scale=1.0)
                s2s[t] = s2
                del crd[t], zs[t]

            def phase_x(t):
                # deferred one iteration behind phase_m so the s2-dependent
                # x matmul never blocks the next chunk's d/e matmuls in the
                # PE's in-order stream
                ht = ht_of(t)
                # x = h + (U2@s2 + b2): PE computes U2@s2, the DVE fused op
                # adds the per-channel bias and the residual in one pass
                x_ps = psA.tile([HID, T], f32, tag="x", bufs=1)
                nc.tensor.matmul(x_ps, sU2, s2s[t], start=True, stop=True)
                base = t * T
                x_sb = x_full[:, base:base + T]
                nc.vector.scalar_tensor_tensor(
                    out=x_sb, in0=x_ps, scalar=sb2, in1=ht,
                    op0=mybir.AluOpType.add, op1=mybir.AluOpType.add)
                x2 = work.tile([HID, T], DT, tag="x2", bufs=2)
                nc.vector.tensor_tensor(out=x2, in0=x_sb, in1=x_sb,
                                        op=mybir.AluOpType.mult)
                # LN stats rows: band-select lhsT packs E[x] into psum row t
                # and E[x^2] into row NCH+t of one accumulating [64,T] bank
                hot = 2 * NCH - 1
                nc.tensor.matmul(st_ps[:, :], ssel[:, hot - t:hot - t + 2 * NCH],
                                 x_sb, start=(t == 0), stop=False)
                nc.tensor.matmul(st_ps[:, :],
                                 ssel[:, hot - NCH - t:hot - t + NCH],
                                 x2, start=False, stop=(t == NCH - 1))
                if 0 < t < NCH - 1:
                    # tiny keep-warm matmuls: the HAM clock gate re-throttles
                    # the PE after idle stretches; these fill the stall tails
                    # so the array stays at full clock (~135ns each)
                    dmy = psB.tile([HID, HID], f32, tag="agg", bufs=1)
                    nc.tensor.matmul(dmy, sW2s, sW2U, start=True, stop=True)
                    dmy2 = psB.tile([HID, HID], f32, tag="agg", bufs=1)
                    nc.tensor.matmul(dmy2, sW2s, sU1a, start=True, stop=True)
                del s2s[t]

            # ---------------- pass 1 (software-pipelined) ----------------
            # PE warm-up: the first ~14us are DMA-bound while h/weights
            # stream in.  A run of back-to-back scratch matmuls (emitted
            # FIRST, so they sit ahead of all real matmuls in the PE's
            # in-order queue) keeps the HAM activity monitor busy so the
            # clock gate is at 8/8 when the real work arrives.
            scr = work.tile([HID, T], DT, tag="scr", bufs=1)
            nc.vector.memset(scr, 0.0)
            for w in range(24):
                if w % 2 == 0:
                    wm = psB.tile([HID, T], f32, tag="agg", bufs=1)
                else:
                    wm = psA.tile([HID, T], f32, tag="u", bufs=2)
                nc.tensor.matmul(wm, scr[:, 0:HID], scr, start=True, stop=True)
            load_chunk(0)
            load_chunk(1)
            load_chunk(2, eng=nc.scalar)
            load_chunk(3, eng=nc.scalar)
            phase_a(0)
            phase_a(1)
            phase_a(2)
            load_fix_consts()
            load_late_consts()
            load_tail_consts()
            phase_e(0)
            # second warm-up burst: during pipeline fill the PE only has the
            # d/e matmuls of chunks 0-2 (~25% duty) and the HAM re-throttles,
            # making iterations 2-7 run at half clock.  These fillers run in
            # the idle window (they only delay matmuls that wait on silu(0)
            # anyway) and keep the activity monitor busy until steady-state
            # density takes over.
            for w in range(26):
                if w % 2 == 0:
                    wm = psB.tile([HID, T], f32, tag="agg", bufs=1)
                else:
                    wm = psA.tile([HID, T], f32, tag="u", bufs=2)
                nc.tensor.matmul(wm, scr[:, 0:HID], scr, start=True, stop=True)
            for i in range(NCH):
                if i + 4 < NCH:
                    load_chunk(i + 4)
                if i + 3 < NCH:
                    phase_a(i + 3)
                # phase_e before phase_x/phase_m: the z-adds reach the DVE
                # queue ahead of the s2-coupled x ops, so silu(i+1) starts
                # as early as possible on ACT
                if i + 1 < NCH:
                    phase_e(i + 1)
                if i >= 1:
                    phase_x(i - 1)
                phase_m(i)
            phase_x(NCH - 1)

            # ---------------- LN stats math ----------------
            # rstd = exp(-0.5 * log(var + eps)) on ACT (ln+exp share a table set)
            r_sb = big.tile([NCH, T], DT)       # rstd per token
            u_sb = big.tile([NCH, T], DT)       # mu * rstd per token
            ex_sb = work.tile([NCH, T], f32, tag="ex")
            nc.vector.tensor_copy(ex_sb, st_ps[0:NCH, :])
            t1 = work.tile([NCH, T], f32, tag="t1")
            nc.vector.tensor_tensor(out=t1, in0=ex_sb, in1=ex_sb,
                                    op=mybir.AluOpType.mult)
            # in-place from here: t1 -> var -> ln(var+eps)
            nc.vector.tensor_tensor(out=t1, in0=st_ps[NCH:2 * NCH, :], in1=t1,
                                    op=mybir.AluOpType.subtract)
            seps = singles.tile([NCH, 1], f32)
            nc.vector.memset(seps, float(EPS))
            szero = singles.tile([NCH, 1], f32)
            nc.vector.memset(szero, 0.0)
            nc.scalar.activation(t1, t1, Log, bias=seps, scale=1.0)
            with nc.allow_low_precision(reason="rstd rows feed fp16 matmuls"):
                nc.scalar.activation(r_sb, t1, Exp, bias=szero, scale=-0.5)
            nc.vector.tensor_tensor(out=u_sb, in0=ex_sb,
                                    in1=r_sb, op=mybir.AluOpType.mult)
            # ---------------- pass 2: normalize ----------------
            # o = x * P1 - P2 with P1 = g x rstd, P2 = g x (mu*rstd) - b x 1
            # built as rank-1 grids on the PE; K=1/K=2 matmul rhs rows are
            # DMA'd down to partition 0 first.
            for t in range(NCH):
                base = t * T
                rr = work.tile([1, T], DT, tag="rr", bufs=4)
                nc.gpsimd.dma_start(out=rr, in_=r_sb[t:t + 1, :])
                uo = work.tile([2, T], DT, tag="uo", bufs=4)
                if t < 4:
                    nc.vector.memset(uo[0:1, :], 1.0)
                nc.sync.dma_start(out=uo[1:2, :], in_=u_sb[t:t + 1, :])
                # rotate through the pass-1 psum banks that are dead in the
                # tail for 4-deep grid pipelining (the chain spans 4 engines,
                # ~4.8us of latency per chunk)
                if t % 4 == 2:
                    p1 = psA.tile([HID, T], f32, tag="d", bufs=1)
                elif t % 4 == 3:
                    p1 = psB.tile([HID, T], f32, tag="agg", bufs=1)
                else:
                    p1 = psA.tile([HID, T], f32, tag="e", bufs=2)
                if t % 2 == 0:
                    p2 = psA.tile([HID, T], f32, tag="u", bufs=2)
                else:
                    p2 = psA.tile([HID, T], f32, tag="x", bufs=1)
                nc.tensor.matmul(p1, sg, rr, start=True, stop=True)
                nc.tensor.matmul(p2, sgnb, uo, start=True, stop=True)
                # p1 -> SBUF via ScalarE so the multiply runs at the fp16 2x
                # DVE rate; the subtract reads p2 from PSUM
                sp1 = work.tile([HID, T], DT, tag="sp1", bufs=4)
                nc.scalar.copy(out=sp1, in_=p1)
                o = opool.tile([HID, T], DT, tag="o", bufs=3)
                nc.vector.tensor_tensor(out=o, in0=x_full[:, base:base + T],
                                        in1=sp1, op=mybir.AluOpType.mult)
                nc.vector.tensor_tensor(out=o, in0=o, in1=p2,
                                        op=mybir.AluOpType.subtract)
                # alternate store queues: gps also carries the rr rows, and
                # the 4MB of output transfers otherwise lag the tail drain
                if t % 2 == 0:
                    nc.gpsimd.dma_start(out=outT[:, base:base + T], in_=o)
                else:
                    nc.scalar.dma_start(out=outT[:, base:base + T], in_=o)

    nc.compile()
    return nc


def _get_compiled(dt_name):
    global _compiled
    if _compiled is None:
        from concourse import mybir
        dt = {"bf16": mybir.dt.bfloat16, "fp16": mybir.dt.float16, "fp32": mybir.dt.float32}[dt_name]
        _compiled = _build_bass(dt)
    return _compiled


DT_NAME = "fp16"


def _sel_band(act_np):
    hot = 2 * NCH - 1
    sel = np.zeros((HID, 2 * 2 * NCH - 1), dtype=np.float32)
    sel[:, hot] = 1.0 / HID
    return sel.astype(act_np)


def kernel(**inputs):
    from concourse.bass_utils import run_bass_kernel_spmd

    h = np.asarray(inputs["h"], dtype=np.float32)
    coord = np.asarray(inputs["coord"], dtype=np.float32)
    msg_w1 = np.asarray(inputs["msg_w1"], dtype=np.float32)
    msg_b1 = np.asarray(inputs["msg_b1"], dtype=np.float32)
    msg_w2 = np.asarray(inputs["msg_w2"], dtype=np.float32)
    msg_b2 = np.asarray(inputs["msg_b2"], dtype=np.float32)
    upd_w1 = np.asarray(inputs["upd_w1"], dtype=np.float32)
    upd_b1 = np.asarray(inputs["upd_b1"], dtype=np.float32)
    upd_w2 = np.asarray(inputs["upd_w2"], dtype=np.float32)
    upd_b2 = np.asarray(inputs["upd_b2"], dtype=np.float32)
    ln_g = np.asarray(inputs["ln_g"], dtype=np.float32)
    ln_b = np.asarray(inputs["ln_b"], dtype=np.float32)

    import ml_dtypes
    act_np = {"bf16": ml_dtypes.bfloat16, "fp16": np.float16, "fp32": np.float32}[DT_NAME]

    W1a = msg_w1[:HID]
    W1b = msg_w1[HID:2 * HID]
    w1c = msg_w1[2 * HID]
    U1b_m = upd_w1[HID:2 * HID]
    bias_u = upd_b1 + msg_b2 @ U1b_m
    W2s = msg_w2 / (2.0 * K)
    W2U = (msg_w2.astype(np.float64) / (2.0 * K) @ U1b_m.astype(np.float64)).astype(np.float32)

    idx = np.arange(N)
    count = (np.minimum(idx, K) + np.minimum(N - 1 - idx, K)).astype(np.float32)
    fix = (2.0 * K) / count
    fixf = fix[:K].reshape(1, K).astype(np.float32)
    fixl = fix[N - K:].reshape(1, K).astype(np.float32)

    const = {
        "W1a": np.ascontiguousarray(W1a, dtype=act_np),
        "W1b": np.ascontiguousarray(W1b, dtype=act_np),
        "w1c": np.ascontiguousarray(w1c.reshape(1, HID), dtype=act_np),
        "w1cn": np.ascontiguousarray(-w1c.reshape(1, HID), dtype=act_np),
        "W2s": np.ascontiguousarray(W2s, dtype=act_np),
        "W2U": np.ascontiguousarray(W2U, dtype=act_np),
        "U1a": np.ascontiguousarray(upd_w1[:HID], dtype=act_np),
        "U1b": np.ascontiguousarray(U1b_m, dtype=act_np),
        "U2": np.ascontiguousarray(upd_w2, dtype=act_np),
        "b1c": np.ascontiguousarray(msg_b1.reshape(HID, 1), dtype=np.float32),
        "buc": np.ascontiguousarray(bias_u.reshape(HID, 1), dtype=np.float32),
        "b2c": np.ascontiguousarray(upd_b2.reshape(HID, 1), dtype=np.float32),
        "g_row": np.ascontiguousarray(ln_g.reshape(1, HID), dtype=act_np),
        "nb_row": np.ascontiguousarray(-ln_b.reshape(1, HID), dtype=act_np),
        "fixf": fixf,
        "fixl": fixl,
        "selb": _sel_band(act_np),
    }

    in_maps = []
    for b in range(B):
        m = dict(const)
        m["hT"] = np.ascontiguousarray(h[b].T, dtype=act_np)
        m["coordR"] = np.ascontiguousarray(coord[b].reshape(1, N), dtype=act_np)
        in_maps.append(m)

    nc = _get_compiled(DT_NAME)
    res = run_bass_kernel_spmd(nc, in_maps, core_ids=list(range(B)))
    global LAST_RESULTS
    LAST_RESULTS = res
    out = np.stack([np.asarray(res.results[b]["outT"], dtype=np.float32).T
                    for b in range(B)])
    return np.ascontiguousarray(out)


# revision 67
# speedup vs baseline: 1.0107x; 1.0012x over previous
"""Trainium2 Bass kernel for the LocalGNOBlock (windowed GNN message passing).

Math restructuring (vs the naive 12x full MLP evaluations):
  msg first layer is linear over concat([h_i, h_j, dc]):
      z_d[i] = (A - C)[i] + (B + C)[i+d] + b1,  d in {+-1..+-6}
  where A = h @ W1a, B = h @ W1b, C = coord x w1c (rank-1).
  The msg second layer AND the update first layer's agg branch are fused:
  agg is only consumed by agg @ U1b, so for interior tokens (count == 12)
      u_pre = h @ U1a + sum_d silu(z_d) @ (W2/12 @ U1b) + bias_u
  accumulates as one 13-matmul PSUM group (no agg materialization at all).
  Boundary chunks (first/last 6 tokens need 12/count fixup) keep the
  two-step path.  LayerNorm stats are computed with band-select matmuls
  (channel dim lives on partitions); rstd = exp(-0.5*ln(var+eps)) on ACT;
  the normalize uses rank-1 grids P1 = g x r, P2 = g x (mu*r) - b x 1.

Pipeline: iteration i emits [load(i+4), phase_a(i+3), phase_e(i+1),
phase_x(i-1), phase_m(i)] so silu(c) (5.4us on ACT, the pacing engine)
completes a full iteration before the matmuls that consume it, and the
s2-dependent x/stats matmuls never block the next chunk's d/e matmuls in
the PE's in-order stream.  Steady-state period ~5.9-6.1us/chunk = the ACT
floor.  Engine balance per chunk: ACT = silu 5.4 + s2 0.7; DVE = z-adds
3.8 + D_A/e casts + x-stt + x2; PE = 20 matmuls; GPSIMD compute idle (it
shares the SBUF port with the DVE - anything on it slows the z-adds) but
its SWDGE queue carries the D_B shift DMAs (AXI port, no engine
contention).  Startup DMAs are spread across the sync/scalar/gpsimd
trigger queues, and a scratch-matmul burst warms the HAM clock gate while
the first h chunks stream in.  The pass-2 tail is a 4-engine chain
(row-DMA -> rank-1 grids on PE -> ScalarE PSUM->SBUF copy -> two DVE ops
-> store) pipelined 4 deep by rotating grids through the pass-1 PSUM
banks that are dead in the tail.

Sharding: batch dim B=8 -> one batch element per NeuronCore.
"""

import numpy as np

K = 6
HID = 128
N = 16384
B = 8
EPS = 1e-5
T = 512                 # token chunk (matmul + elementwise granularity)
NCH = N // T            # 32 chunks
OFF0 = 8                # D_full column of token 0 (even, for alignment)
NCOL = N + 2 * OFF0     # D_full width

# offsets ordered in 4 stride-2 groups: (even uses D_A, odd uses D_B)
NEG_EVEN = [-6, -4, -2]
NEG_ODD = [-5, -3, -1]
POS_ODD = [1, 3, 5]
POS_EVEN = [2, 4, 6]
SEG_ORDER = NEG_EVEN + NEG_ODD + POS_ODD + POS_EVEN  # 12 segments in Z

_compiled = None


def _build_bass(dt_act):
    import concourse.bacc as bacc
    import concourse.bass as bass
    import concourse.tile as tile
    from concourse import mybir

    f32 = mybir.dt.float32
    DT = dt_act

    nc = bacc.Bacc("TRN2", target_bir_lowering=False, debug=False)

    # ---- DRAM I/O ----
    hT = nc.dram_tensor("hT", [HID, N], DT, kind="ExternalInput")
    coordR = nc.dram_tensor("coordR", [1, N], DT, kind="ExternalInput")
    W1a = nc.dram_tensor("W1a", [HID, HID], DT, kind="ExternalInput")
    W1b = nc.dram_tensor("W1b", [HID, HID], DT, kind="ExternalInput")
    w1c = nc.dram_tensor("w1c", [1, HID], DT, kind="ExternalInput")      # +w1c
    w1cn = nc.dram_tensor("w1cn", [1, HID], DT, kind="ExternalInput")    # -w1c
    W2s = nc.dram_tensor("W2s", [HID, HID], DT, kind="ExternalInput")     # W2/12
    W2U = nc.dram_tensor("W2U", [HID, HID], DT, kind="ExternalInput")     # W2/12 @ U1b
    U1a = nc.dram_tensor("U1a", [HID, HID], DT, kind="ExternalInput")
    U1b = nc.dram_tensor("U1b", [HID, HID], DT, kind="ExternalInput")
    U2 = nc.dram_tensor("U2", [HID, HID], DT, kind="ExternalInput")
    b1c = nc.dram_tensor("b1c", [HID, 1], f32, kind="ExternalInput")      # msg_b1
    buc = nc.dram_tensor("buc", [HID, 1], f32, kind="ExternalInput")      # upd_b1 + b2@U1b
    b2c = nc.dram_tensor("b2c", [HID, 1], f32, kind="ExternalInput")      # upd_b2 col
    g_row = nc.dram_tensor("g_row", [1, HID], DT, kind="ExternalInput")  # ln_g
    nb_row = nc.dram_tensor("nb_row", [1, HID], DT, kind="ExternalInput")  # -ln_b
    fixf = nc.dram_tensor("fixf", [1, K], f32, kind="ExternalInput")      # 12/count head
    fixl = nc.dram_tensor("fixl", [1, K], f32, kind="ExternalInput")      # 12/count tail
    # band-select matrix: column 63 = 1/128, else 0 (stats row packing)
    selb = nc.dram_tensor("selb", [HID, 2 * 2 * NCH - 1], DT, kind="ExternalInput")
    outT = nc.dram_tensor("outT", [HID, N], DT, kind="ExternalOutput")
    # DRAM bounce rows for the pass-2 broadcast loads (SBUF sources cannot
    # have a stride-0 partition AP, DRAM sources can)
    rN = nc.dram_tensor("rN", [1, N], DT, kind="Internal")
    uN = nc.dram_tensor("uN", [1, N], DT, kind="Internal")

    Silu = mybir.ActivationFunctionType.Silu
    Log = mybir.ActivationFunctionType.Ln
    Exp = mybir.ActivationFunctionType.Exp

    with tile.TileContext(nc) as tc:
        with (
            tc.tile_pool(name="singles", bufs=1) as singles,
            tc.tile_pool(name="big", bufs=1) as big,
            tc.tile_pool(name="work", bufs=2) as work,
            tc.tile_pool(name="zpool", bufs=3) as zpool,
            tc.tile_pool(name="opool", bufs=2) as opool,
            tc.tile_pool(name="psA", bufs=1, space="PSUM") as psA,
            tc.tile_pool(name="psB", bufs=1, space="PSUM") as psB,
            tc.tile_pool(name="psS", bufs=1, space="PSUM") as psS,
        ):
            # ---- constants into SBUF ----
            # the tensors phase_a(0)/phase_e(0) need go on the queue FIRST so
            # the pipeline starts as soon as chunk 0 arrives
            sW1a = singles.tile([HID, HID], DT)
            sW1b = singles.tile([HID, HID], DT)
            sW2s = singles.tile([HID, HID], DT)
            sW2U = singles.tile([HID, HID], DT)
            sU1a = singles.tile([HID, HID], DT)
            sU1b = singles.tile([HID, HID], DT)
            sU2 = singles.tile([HID, HID], DT)
            sw1c = singles.tile([1, HID], DT)
            sw1cn = singles.tile([1, HID], DT)
            sb1 = singles.tile([HID, 1], f32)
            sbu = singles.tile([HID, 1], f32)
            sb2 = singles.tile([HID, 1], f32)
            # phase_a needs: W1b, w1c (sync queue); phase_e needs: W1a, w1cn,
            # b1c (scalar queue - ScalarE is a HWDGE engine too and is idle
            # at startup); this leaves the sync queue free for the h loads
            nc.sync.dma_start(out=sW1b, in_=W1b[:, :])
            nc.sync.dma_start(out=sw1c, in_=w1c[:, :])
            nc.scalar.dma_start(out=sW1a, in_=W1a[:, :])
            nc.scalar.dma_start(out=sw1cn, in_=w1cn[:, :])
            nc.scalar.dma_start(out=sb1, in_=b1c[:, :])

            def load_late_consts():
                # everything first needed from phase_m(0) onwards, on the
                # scalar queue which idles until the first silu
                for sb, dr in [(sW2s, W2s), (sW2U, W2U),
                               (sU1a, U1a), (sU1b, U1b), (sU2, U2)]:
                    nc.scalar.dma_start(out=sb, in_=dr[:, :])
                nc.scalar.dma_start(out=sbu, in_=buc[:, :])
                nc.scalar.dma_start(out=sb2, in_=b2c[:, :])
            # broadcast [1,6] -> [128,6] fix tiles
            sfixf = singles.tile([HID, K], f32)
            sfixl = singles.tile([HID, K], f32)
            def bcast_rows(dr):
                a = dr[0:1, :]
                return bass.AP(tensor=a.tensor, offset=a.offset,
                               ap=[[0, HID]] + list(a.ap[1:]))

            def load_fix_consts():
                # broadcast loads must use the gpsimd SWDGE queue (HWDGE
                # rejects stride-0 partition APs); emitted after the D_B
                # copies for chunks 0-2 so those aren't queued behind them
                nc.gpsimd.dma_start(out=sfixf, in_=bcast_rows(fixf))
                nc.gpsimd.dma_start(out=sfixl, in_=bcast_rows(fixl))
            ssel = singles.tile([HID, 2 * 2 * NCH - 1], DT)
            # [-b ; g] stacked lhsT and [ones ; uu] stacked rhs let p2 be a
            # single K=2 matmul in the tail
            sgnb = singles.tile([2, HID], DT)
            sg = singles.tile([1, HID], DT)

            def load_tail_consts():
                nc.scalar.dma_start(out=ssel, in_=selb[:, :])
                nc.scalar.dma_start(out=sgnb[0:1, :], in_=nb_row[:, :])
                nc.scalar.dma_start(out=sgnb[1:2, :], in_=g_row[:, :])
                nc.scalar.dma_start(out=sg, in_=g_row[:, :])

            # ---- big persistent buffers ----
            h_full = big.tile([HID, N], DT)
            D_A = big.tile([HID, NCOL], DT)      # token j at col OFF0 + j
            D_B = big.tile([HID, NCOL], DT)      # token j at col OFF0 + 1 + j
            x_full = big.tile([HID, N], DT)
            # zero halo columns of D so boundary silu stays finite
            nc.vector.memset(D_A[:, 0:OFF0], 0.0)
            nc.vector.memset(D_A[:, OFF0 + N:NCOL], 0.0)
            nc.vector.memset(D_B[:, 0:OFF0 + 1], 0.0)
            nc.vector.memset(D_B[:, OFF0 + 1 + N:NCOL], 0.0)

            # LN stats: rows [0:32] = E[x]/chunk, [32:64] = E[x^2]/chunk
            st_ps = psS.tile([2 * NCH, T], f32)

            crd = {}
            zs = {}
            s2s = {}

            def ht_of(c):
                return h_full[:, c * T:(c + 1) * T]

            def load_chunk(c, eng=None):
                q = eng if eng is not None else nc.sync
                q.dma_start(out=h_full[:, c * T:(c + 1) * T],
                            in_=hT[:, c * T:(c + 1) * T])
                co = work.tile([1, T], DT, tag="co", bufs=5)
                q.dma_start(out=co, in_=coordR[:, c * T:(c + 1) * T])
                crd[c] = co

            def phase_a(c):
                # D chunk = W1b.T @ h  +  w1c x coord   (PSUM accumulate)
                d_ps = psA.tile([HID, T], f32, tag="d", bufs=1)
                nc.tensor.matmul(d_ps, sW1b, ht_of(c), start=True, stop=False)
                nc.tensor.matmul(d_ps, sw1c, crd[c], start=False, stop=True)
                col = OFF0 + c * T
                nc.vector.tensor_copy(D_A[:, col:col + T], d_ps)
                # shifted copy for odd-offset alignment: DMA uses the AXI
                # port, so it does not contend with DVE/ACT engine ports;
                # the gpsimd queue keeps it off the sync queue's h loads
                nc.gpsimd.dma_start(out=D_B[:, col + 1:col + 1 + T],
                                    in_=D_A[:, col:col + T])

            def seg_in1(tile_ap, col):
                # [128, 3, T] AP over D with outer column-stride 2
                s = tile_ap[:, col:col + T]
                return bass.AP(tensor=s.tensor, offset=s.offset,
                               ap=[s.ap[0], [2, 3], [1, T]])

            def phase_e(t):
                # E chunk = W1a.T @ h - w1c x coord
                e_ps = psA.tile([HID, T], f32, tag="e", bufs=2)
                nc.tensor.matmul(e_ps, sW1a, ht_of(t), start=True, stop=False)
                nc.tensor.matmul(e_ps, sw1cn, crd[t], start=False, stop=True)
                e_sb = work.tile([HID, T], DT, tag="esb", bufs=2)
                nc.vector.tensor_copy(e_sb, e_ps)

                # Z: 12 segments of E + shifted D, 4 stride-2 groups
                z = zpool.tile([HID, 12 * T], DT, tag="z", bufs=3)
                zv = z.rearrange("p (s t) -> p s t", t=T)
                e_b = bass.AP(tensor=e_sb.tensor, offset=e_sb.offset,
                              ap=[e_sb.ap[0], [0, 3], [1, T]])
                base = t * T
                groups = [
                    (D_A, OFF0 + base + NEG_EVEN[0]),
                    (D_B, OFF0 + 1 + base + NEG_ODD[0]),
                    (D_B, OFF0 + 1 + base + POS_ODD[0]),
                    (D_A, OFF0 + base + POS_EVEN[0]),
                ]
                for gi, (dbuf, col) in enumerate(groups):
                    nc.vector.tensor_tensor(
                        out=zv[:, 3 * gi:3 * gi + 3, :],
                        in0=e_b, in1=seg_in1(dbuf, col),
                        op=mybir.AluOpType.add)

                # silu over all 12 segments at once (bias = msg_b1)
                nc.scalar.activation(z, z, Silu, bias=sb1, scale=1.0)

                # zero invalid boundary columns (torn edges of the sequence)
                if t == 0:
                    for s, d in enumerate(SEG_ORDER):
                        if d < 0:
                            nc.vector.memset(zv[:, s, 0:-d], 0.0)
                if t == NCH - 1:
                    for s, d in enumerate(SEG_ORDER):
                        if d > 0:
                            nc.vector.memset(zv[:, s, T - d:T], 0.0)
                zs[t] = z

            def phase_m(t):
                ht = ht_of(t)
                zv = zs[t].rearrange("p (s t) -> p s t", t=T)
                boundary = t == 0 or t == NCH - 1
                u_ps = psA.tile([HID, T], f32, tag="u", bufs=2)
                if boundary:
                    # two-step path so the 12/count fixup can apply to agg
                    a_ps = psB.tile([HID, T], f32, tag="agg", bufs=1)
                    for s in range(12):
                        nc.tensor.matmul(a_ps, sW2s, zv[:, s, :],
                                         start=(s == 0), stop=(s == 11))
                    agg = work.tile([HID, T], DT, tag="agg_sb", bufs=1)
                    nc.vector.tensor_copy(agg, a_ps)
                    if t == 0:
                        nc.vector.tensor_tensor(
                            out=agg[:, 0:K], in0=a_ps[:, 0:K],
                            in1=sfixf, op=mybir.AluOpType.mult)
                    else:
                        nc.vector.tensor_tensor(
                            out=agg[:, T - K:T], in0=a_ps[:, T - K:T],
                            in1=sfixl, op=mybir.AluOpType.mult)
                    nc.tensor.matmul(u_ps, sU1a, ht, start=True, stop=False)
                    nc.tensor.matmul(u_ps, sU1b, agg, start=False, stop=True)
                else:
                    # fused: u_pre = U1a.T@h + sum_s W2U.T@silu(z_s)
                    nc.tensor.matmul(u_ps, sU1a, ht, start=True, stop=False)
                    for s in range(12):
                        nc.tensor.matmul(u_ps, sW2U, zv[:, s, :],
                                         start=False, stop=(s == 11))
                s2 = work.tile([HID, T], DT, tag="s2", bufs=2)
                nc.scalar.activation(s2, u_ps, Silu, bias=sbu, scale=1.0)
                s2s[t] = s2
                del crd[t], zs[t]

            def phase_x(t):
                # deferred one iteration behind phase_m so the s2-dependent
                # x matmul never blocks the next chunk's d/e matmuls in the
                # PE's in-order stream
                ht = ht_of(t)
                # x = h + (U2@s2 + b2): PE computes U2@s2, the DVE fused op
                # adds the per-channel bias and the residual in one pass
                x_ps = psA.tile([HID, T], f32, tag="x", bufs=1)
                nc.tensor.matmul(x_ps, sU2, s2s[t], start=True, stop=True)
                base = t * T
                x_sb = x_full[:, base:base + T]
                nc.vector.scalar_tensor_tensor(
                    out=x_sb, in0=x_ps, scalar=sb2, in1=ht,
                    op0=mybir.AluOpType.add, op1=mybir.AluOpType.add)
                x2 = work.tile([HID, T], DT, tag="x2", bufs=2)
                nc.vector.tensor_tensor(out=x2, in0=x_sb, in1=x_sb,
                                        op=mybir.AluOpType.mult)
                # LN stats rows: band-select lhsT packs E[x] into psum row t
                # and E[x^2] into row NCH+t of one accumulating [64,T] bank
                hot = 2 * NCH - 1
                nc.tensor.matmul(st_ps[:, :], ssel[:, hot - t:hot - t + 2 * NCH],
                                 x_sb, start=(t == 0), stop=False)
                nc.tensor.matmul(st_ps[:, :],
                                 ssel[:, hot - NCH - t:hot - t + NCH],
                                 x2, start=False, stop=(t == NCH - 1))
                if 0 < t < NCH - 1:
                    # tiny keep-warm matmuls: the HAM clock gate re-throttles
                    # the PE after idle stretches; these fill the stall tails
                    # so the array stays at full clock (~135ns each)
                    dmy = psB.tile([HID, HID], f32, tag="agg", bufs=1)
                    nc.tensor.matmul(dmy, sW2s, sW2U, start=True, stop=True)
                    dmy2 = psB.tile([HID, HID], f32, tag="agg", bufs=1)
                    nc.tensor.matmul(dmy2, sW2s, sU1a, start=True, stop=True)
                del s2s[t]

            # ---------------- pass 1 (software-pipelined) ----------------
            # PE warm-up: the first ~14us are DMA-bound while h/weights
            # stream in.  A run of back-to-back scratch matmuls (emitted
            # FIRST, so they sit ahead of all real matmuls in the PE's
            # in-order queue) keeps the HAM activity monitor busy so the
            # clock gate is at 8/8 when the real work arrives.
            scr = work.tile([HID, T], DT, tag="scr", bufs=1)
            nc.vector.memset(scr, 0.0)
            for w in range(24):
                if w % 2 == 0:
                    wm = psB.tile([HID, T], f32, tag="agg", bufs=1)
                else:
                    wm = psA.tile([HID, T], f32, tag="u", bufs=2)
                nc.tensor.matmul(wm, scr[:, 0:HID], scr, start=True, stop=True)
            load_chunk(0)
            load_chunk(1)
            load_chunk(2, eng=nc.scalar)
            load_chunk(3, eng=nc.scalar)
            phase_a(0)
            phase_a(1)
            phase_a(2)
            load_fix_consts()
            load_late_consts()
            load_tail_consts()
            phase_e(0)
            # second warm-up burst: during pipeline fill the PE only has the
            # d/e matmuls of chunks 0-2 (~25% duty) and the HAM re-throttles,
            # making iterations 2-7 run at half clock.  These fillers run in
            # the idle window (they only delay matmuls that wait on silu(0)
            # anyway) and keep the activity monitor busy until steady-state
            # density takes over.
            for w in range(16):
                if w % 2 == 0:
                    wm = psB.tile([HID, T], f32, tag="agg", bufs=1)
                else:
                    wm = psA.tile([HID, T], f32, tag="u", bufs=2)
                nc.tensor.matmul(wm, scr[:, 0:HID], scr, start=True, stop=True)
            for i in range(NCH):
                if i + 4 < NCH:
                    load_chunk(i + 4)
                if i + 3 < NCH:
                    phase_a(i + 3)
                # phase_e before phase_x/phase_m: the z-adds reach the DVE
                # queue ahead of the s2-coupled x ops, so silu(i+1) starts
                # as early as possible on ACT
                if i + 1 < NCH:
                    phase_e(i + 1)
                if i >= 1:
                    phase_x(i - 1)
                phase_m(i)
            phase_x(NCH - 1)

            # ---------------- LN stats math ----------------
            # rstd = exp(-0.5 * log(var + eps)) on ACT (ln+exp share a table set)
            r_sb = big.tile([NCH, T], DT)       # rstd per token
            u_sb = big.tile([NCH, T], DT)       # mu * rstd per token
            ex_sb = work.tile([NCH, T], f32, tag="ex")
            nc.vector.tensor_copy(ex_sb, st_ps[0:NCH, :])
            t1 = work.tile([NCH, T], f32, tag="t1")
            nc.vector.tensor_tensor(out=t1, in0=ex_sb, in1=ex_sb,
                                    op=mybir.AluOpType.mult)
            # in-place from here: t1 -> var -> ln(var+eps)
            nc.vector.tensor_tensor(out=t1, in0=st_ps[NCH:2 * NCH, :], in1=t1,
                                    op=mybir.AluOpType.subtract)
            seps = singles.tile([NCH, 1], f32)
            nc.vector.memset(seps, float(EPS))
            szero = singles.tile([NCH, 1], f32)
            nc.vector.memset(szero, 0.0)
            nc.scalar.activation(t1, t1, Log, bias=seps, scale=1.0)
            with nc.allow_low_precision(reason="rstd rows feed fp16 matmuls"):
                nc.scalar.activation(r_sb, t1, Exp, bias=szero, scale=-0.5)
            nc.vector.tensor_tensor(out=u_sb, in0=ex_sb,
                                    in1=r_sb, op=mybir.AluOpType.mult)
            # ---------------- pass 2: normalize ----------------
            # o = x * P1 - P2 with P1 = g x rstd, P2 = g x (mu*rstd) - b x 1
            # built as rank-1 grids on the PE; K=1/K=2 matmul rhs rows are
            # DMA'd down to partition 0 first.
            for t in range(NCH):
                base = t * T
                rr = work.tile([1, T], DT, tag="rr", bufs=4)
                nc.gpsimd.dma_start(out=rr, in_=r_sb[t:t + 1, :])
                uo = work.tile([2, T], DT, tag="uo", bufs=4)
                if t < 4:
                    nc.vector.memset(uo[0:1, :], 1.0)
                nc.sync.dma_start(out=uo[1:2, :], in_=u_sb[t:t + 1, :])
                # rotate through the pass-1 psum banks that are dead in the
                # tail for 4-deep grid pipelining (the chain spans 4 engines,
                # ~4.8us of latency per chunk)
                if t % 4 == 2:
                    p1 = psA.tile([HID, T], f32, tag="d", bufs=1)
                elif t % 4 == 3:
                    p1 = psB.tile([HID, T], f32, tag="agg", bufs=1)
                else:
                    p1 = psA.tile([HID, T], f32, tag="e", bufs=2)
                if t % 2 == 0:
                    p2 = psA.tile([HID, T], f32, tag="u", bufs=2)
                else:
                    p2 = psA.tile([HID, T], f32, tag="x", bufs=1)
                nc.tensor.matmul(p1, sg, rr, start=True, stop=True)
                nc.tensor.matmul(p2, sgnb, uo, start=True, stop=True)
                # p1 -> SBUF via ScalarE so the multiply runs at the fp16 2x
                # DVE rate; the subtract reads p2 from PSUM
                sp1 = work.tile([HID, T], DT, tag="sp1", bufs=4)
                nc.scalar.copy(out=sp1, in_=p1)
                o = opool.tile([HID, T], DT, tag="o", bufs=3)
                nc.vector.tensor_tensor(out=o, in0=x_full[:, base:base + T],
                                        in1=sp1, op=mybir.AluOpType.mult)
                nc.vector.tensor_tensor(out=o, in0=o, in1=p2,
                                        op=mybir.AluOpType.subtract)
                # alternate store queues: gps also carries the rr rows, and
                # the 4MB of output transfers otherwise lag the tail drain
                if t % 2 == 0:
                    nc.gpsimd.dma_start(out=outT[:, base:base + T], in_=o)
                else:
                    nc.scalar.dma_start(out=outT[:, base:base + T], in_=o)

    nc.compile()
    return nc


def _get_compiled(dt_name):
    global _compiled
    if _compiled is None:
        from concourse import mybir
        dt = {"bf16": mybir.dt.bfloat16, "fp16": mybir.dt.float16, "fp32": mybir.dt.float32}[dt_name]
        _compiled = _build_bass(dt)
    return _compiled


DT_NAME = "fp16"


def _sel_band(act_np):
    hot = 2 * NCH - 1
    sel = np.zeros((HID, 2 * 2 * NCH - 1), dtype=np.float32)
    sel[:, hot] = 1.0 / HID
    return sel.astype(act_np)


def kernel(**inputs):
    from concourse.bass_utils import run_bass_kernel_spmd

    h = np.asarray(inputs["h"], dtype=np.float32)
    coord = np.asarray(inputs["coord"], dtype=np.float32)
    msg_w1 = np.asarray(inputs["msg_w1"], dtype=np.float32)
    msg_b1 = np.asarray(inputs["msg_b1"], dtype=np.float32)
    msg_w2 = np.asarray(inputs["msg_w2"], dtype=np.float32)
    msg_b2 = np.asarray(inputs["msg_b2"], dtype=np.float32)
    upd_w1 = np.asarray(inputs["upd_w1"], dtype=np.float32)
    upd_b1 = np.asarray(inputs["upd_b1"], dtype=np.float32)
    upd_w2 = np.asarray(inputs["upd_w2"], dtype=np.float32)
    upd_b2 = np.asarray(inputs["upd_b2"], dtype=np.float32)
    ln_g = np.asarray(inputs["ln_g"], dtype=np.float32)
    ln_b = np.asarray(inputs["ln_b"], dtype=np.float32)

    import ml_dtypes
    act_np = {"bf16": ml_dtypes.bfloat16, "fp16": np.float16, "fp32": np.float32}[DT_NAME]

    W1a = msg_w1[:HID]
    W1b = msg_w1[HID:2 * HID]
    w1c = msg_w1[2 * HID]
    U1b_m = upd_w1[HID:2 * HID]
    bias_u = upd_b1 + msg_b2 @ U1b_m
    W2s = msg_w2 / (2.0 * K)
    W2U = (msg_w2.astype(np.float64) / (2.0 * K) @ U1b_m.astype(np.float64)).astype(np.float32)

    idx = np.arange(N)
    count = (np.minimum(idx, K) + np.minimum(N - 1 - idx, K)).astype(np.float32)
    fix = (2.0 * K) / count
    fixf = fix[:K].reshape(1, K).astype(np.float32)
    fixl = fix[N - K:].reshape(1, K).astype(np.float32)

    const = {
        "W1a": np.ascontiguousarray(W1a, dtype=act_np),
        "W1b": np.ascontiguousarray(W1b, dtype=act_np),
        "w1c": np.ascontiguousarray(w1c.reshape(1, HID), dtype=act_np),
        "w1cn": np.ascontiguousarray(-w1c.reshape(1, HID), dtype=act_np),
        "W2s": np.ascontiguousarray(W2s, dtype=act_np),
        "W2U": np.ascontiguousarray(W2U, dtype=act_np),
        "U1a": np.ascontiguousarray(upd_w1[:HID], dtype=act_np),
        "U1b": np.ascontiguousarray(U1b_m, dtype=act_np),
        "U2": np.ascontiguousarray(upd_w2, dtype=act_np),
        "b1c": np.ascontiguousarray(msg_b1.reshape(HID, 1), dtype=np.float32),
        "buc": np.ascontiguousarray(bias_u.reshape(HID, 1), dtype=np.float32),
        "b2c": np.ascontiguousarray(upd_b2.reshape(HID, 1), dtype=np.float32),
        "g_row": np.ascontiguousarray(ln_g.reshape(1, HID), dtype=act_np),
        "nb_row": np.ascontiguousarray(-ln_b.reshape(1, HID), dtype=act_np),
        "fixf": fixf,
        "fixl": fixl,
        "selb": _sel_band(act_np),
    }

    in_maps = []
    for b in range(B):
        m = dict(const)
        m["hT"] = np.ascontiguousarray(h[b].T, dtype=act_np)
        m["coordR"] = np.ascontiguousarray(coord[b].reshape(1, N), dtype=act_np)
        in_maps.append(m)

    nc = _get_compiled(DT_NAME)
    res = run_bass_kernel_spmd(nc, in_maps, core_ids=list(range(B)))
    global LAST_RESULTS
    LAST_RESULTS = res
    out = np.stack([np.asarray(res.results[b]["outT"], dtype=np.float32).T
                    for b in range(B)])
    return np.ascontiguousarray(out)
